# revision 2
# baseline (speedup 1.0000x reference)
"""Causal self-attention head (softmax over the QUERY axis) on 8 trn2 cores.

Reference math (softmax axis=-2, i.e. per key-column):
    q = x @ Wq; k = x @ Wk; v = x @ Wv            # [B,T,64]
    s[b,q,k] = (q . k) * 64**-0.5, masked to q >= k
    w[:, k]  = softmax over q of s[:, k]           # column softmax
    out[b,q,:] = sum_k w[q,k] v[k,:]

The softmax normalizes over q, so the normalizer folds into per-key scaling:
    out[q] = sum_{k<=q} exp(s[q,k]) * (r[k] * v[k]),  r[k] = 1/sum_{q>=k} exp(s[q,k])

Sharding: 8 cores = 4 batches x 2 "parities". Core (b, p) owns key blocks
2i+p (128 keys each); parity-1 cores get x^T pre-shifted by 128 cols
(zero-pad tail killed by a tailmask matmul); host folds + shifts output back.

v3 kernel structure (per core, pairs j = 7..0, pair = key blocks 2j/2j+1):
- proj: ONE fused [Wq||Wk] matmul per contraction subtile: psum rows 0-63 =
  q, rows 64-127 = k for all 512 chunk cols. q evacuated to qT2[0:64] (DVE)
  and duplicated to qT2[64:128] by an SBUF->SBUF DMA; k-odd evacuated to
  kT2[64:128] (DVE), k-even staged + DMA'd down to kT2[0:64].
- v projected into natural [key, ch] layout (lhsT = x-chunk key cols).
- scores: ROW-TILED pairs: even block as PE tile (0,0) (contraction rows
  0-63), odd block as tile (64,0) (rows 64-127) -> both 512-col streams run
  CONCURRENTLY (2x). Separate single-buffered psum groups per chain
  (ETILE=1024); causal diag via triangular-count matmul; exp on ACT.
- colsums: DVE reduce over each block's full w2 row (masked entries exp to
  0); pair 0 uses ACT accum_out instead to shorten the tail.
- output: regions accumulate TWO pairs per psum tile (batch k = pairs
  {2k+1,2k}, col-tiled even/odd halves), then one DVE op into the fp32
  outacc; the final batch emits bf16 and streams the DMA out.
"""

import os
import sys
from math import ceil

import numpy as np

for _p in ("/opt/trn_rl_repo",):
    if _p not in sys.path:
        sys.path.insert(0, _p)

import concourse.bass as bass
import concourse.mybir as mybir
from concourse import bacc
from concourse.bass_utils import run_bass_kernel_spmd
from concourse.tile import TileContext

B, T, CE, CH = 4, 4096, 1024, 64
P = 128
NB = 16          # key blocks per core (128 keys each)
NP = 8           # pairs (512-col chunks)
SCALE = CH ** -0.5
NEG = -1e30
M0 = NEG / P     # per-unit magnitude for the triangular-count mask
ETILE = 1024     # scores psum group width (2 banks)

F32 = mybir.dt.float32
BF16 = mybir.dt.bfloat16

N_CORES = 8

LAST_RESULTS = None


def _build_program():
    nc = bacc.Bacc("TRN2", target_bir_lowering=False, debug=False)

    # x pre-permuted on the host: xpre[j, p, o*512+f] = x^T[o*128+p, 512j+f]
    xpre = nc.declare_dram_parameter("xpre", [NP, P, CE // P * 512], BF16, isOutput=False)
    wqk = nc.declare_dram_parameter("wqk", [CE, P], BF16, isOutput=False)
    wv = nc.declare_dram_parameter("wv", [CE, CH], BF16, isOutput=False)
    tailmask = nc.declare_dram_parameter("tailmask", [P, P], BF16, isOutput=False)
    outT = nc.declare_dram_parameter("outT", [P, T], BF16, isOutput=True)

    with TileContext(nc) as tc:
        with (
            tc.tile_pool(name="consts", bufs=1) as consts,
            tc.tile_pool(name="qkv", bufs=1) as qkv,
            tc.tile_pool(name="w2p", bufs=1) as w2p,
            tc.tile_pool(name="xp", bufs=3) as xp,
            tc.tile_pool(name="ksp", bufs=2) as ksp,
            tc.tile_pool(name="osb", bufs=2) as osb,
            tc.tile_pool(name="ppqk", bufs=1, space="PSUM") as ppqk,
            tc.tile_pool(name="ppv", bufs=1, space="PSUM") as ppv,
            tc.tile_pool(name="spe", bufs=1, space="PSUM") as spe,
            tc.tile_pool(name="spo", bufs=1, space="PSUM") as spo,
            tc.tile_pool(name="op", bufs=2, space="PSUM") as op,
        ):
            # ---- gpsimd-built mask constants (ones FIRST: warmup dep) ----
            ones = consts.tile([P, P], BF16, tag="ones")
            nc.gpsimd.memset(ones[:], 1.0)
            atri = consts.tile([P, P], BF16, tag="atri")
            nc.gpsimd.memset(atri[:], 1.0)
            nc.gpsimd.affine_select(
                out=atri[:],
                in_=atri[:],
                compare_op=mybir.AluOpType.is_ge,
                fill=0.0,
                base=-1,
                pattern=[[1, P]],
                channel_multiplier=-1,
            )
            bneg = consts.tile([P, P], BF16, tag="bneg")
            nc.gpsimd.memset(bneg[:], M0)
            nc.gpsimd.affine_select(
                out=bneg[:],
                in_=bneg[:],
                compare_op=mybir.AluOpType.is_ge,
                fill=0.0,
                base=0,
                pattern=[[-1, P]],
                channel_multiplier=1,
            )

            # ---- DMA'd constants ----
            wqk_sb = consts.tile([P, CE // P, P], BF16, tag="wqk")
            wv_sb = consts.tile([P, CE // P, CH], BF16, tag="wv")
            nc.sync.dma_start(wqk_sb[:], wqk.rearrange("(o p) f -> p o f", p=P))
            nc.sync.dma_start(wv_sb[:], wv.rearrange("(o p) f -> p o f", p=P))
            tmask = consts.tile([P, P], BF16, tag="tmask")
            nc.sync.dma_start(tmask[:], tailmask[:])

            # ---- persistent activations ----
            qT2 = qkv.tile([P, T], BF16, tag="qT2")        # q at rows 0-63 AND 64-127
            kT2 = qkv.tile([P, NP * P], BF16, tag="kT2")   # even k rows 0-63, odd rows 64-127
            vnat = qkv.tile([P, NB, CH], F32, tag="vnat")
            vsc = qkv.tile([P, NB, CH], BF16, tag="vsc")
            stats = qkv.tile([P, 2, 4], F32, tag="stats")  # pair-0 accum_out
            ssum = qkv.tile([P, NB], F32, tag="ssum")
            rr = qkv.tile([P, NB], F32, tag="rr")
            outacc = qkv.tile([P, T], F32, tag="outacc")

            w2 = [
                w2p.tile([P, T - 512 * (i // 2)], BF16, tag=f"w2_{i}", name=f"w2_{i}")
                for i in range(NB)
            ]

            # PE warm-up while the first DMAs land (ones is the first gpsimd
            # memset, so this starts as soon as the preamble ends).
            for t in range(28):
                dscr = op.tile([P, 512], F32, tag="po", name=f"warm{t}")
                nc.tensor.matmul(
                    dscr[:, 0:P], ones[:, 0:P], ones[:, 0:P],
                    start=True, stop=True,
                )
            dscr = op.tile([P, 512], F32, tag="po", name="abs_tm")
            nc.tensor.matmul(
                dscr[0:1, 0:1], tmask[:, 0:1], tmask[:, 0:1],
                start=True, stop=True,
            )

            # ---- projection for chunk j, split into 3 emission parts ----
            def proj_A(j, xtile):
                nc.sync.dma_start(
                    xtile[:, 0:4, :],
                    xpre[j, :, 0 : 4 * 512].rearrange("p (o f) -> p o f", o=4),
                )
                nc.gpsimd.dma_start(
                    xtile[:, 4:8, :],
                    xpre[j, :, 4 * 512 : 8 * 512].rearrange("p (o f) -> p o f", o=4),
                )
                dscr = op.tile([P, 512], F32, tag="po", name=f"absx{j}")
                nc.tensor.matmul(
                    dscr[0:1, 0:1],
                    xtile[:, 0, 0:1],
                    xtile[:, 0, 0:1],
                    start=True,
                    stop=True,
                )
                qkps = ppqk.tile([P, 512], F32, tag="qkps")
                for s in range(CE // P):
                    nc.tensor.matmul(
                        qkps[:],
                        wqk_sb[:, s, :],
                        xtile[:, s, :],
                        start=(s == 0),
                        stop=(s == CE // P - 1),
                    )
                return qkps

            def proj_B(j, xtile):
                vps = ppv.tile([P, P], F32, tag="vps", name=f"vps{j}")
                for s in range(CE // P):
                    nc.tensor.matmul(
                        vps[:, 0:CH],
                        xtile[:, s, 0:P],
                        wv_sb[:, s, :],
                        start=(s == 0),
                        stop=(s == CE // P - 1),
                        skip_group_check=True,
                    )
                for s in range(CE // P):
                    nc.tensor.matmul(
                        vps[:, CH:P],
                        xtile[:, s, 256 : 256 + P],
                        wv_sb[:, s, :],
                        start=(s == 0),
                        stop=(s == CE // P - 1),
                        skip_group_check=True,
                    )
                return vps

            def proj_C(j, qkps, vps):
                # q -> rows 0-63, then dup to rows 64-127 (SBUF->SBUF DMA)
                nc.vector.tensor_copy(qT2[0:CH, 512 * j : 512 * (j + 1)], qkps[0:CH, :])
                nc.sync.dma_start(
                    qT2[CH:P, 512 * j : 512 * (j + 1)],
                    qT2[0:CH, 512 * j : 512 * (j + 1)],
                )
                # k-odd (chunk cols 256:384) -> kT2 rows 64-127 directly
                nc.vector.tensor_copy(
                    kT2[CH:P, P * j : P * (j + 1)], qkps[CH:P, 256:384]
                )
                # k-even (chunk cols 0:128): stage at rows 64-127, DMA down
                kstg = ksp.tile([P, P], BF16, tag="kstg")
                nc.vector.tensor_copy(kstg[CH:P, :], qkps[CH:P, 0:P])
                nc.sync.dma_start(kT2[0:CH, P * j : P * (j + 1)], kstg[CH:P, :])
                # v blocks -> natural layout
                nc.vector.tensor_copy(
                    vnat[:, 2 * j : 2 * j + 2, :].rearrange("p b c -> p (b c)"),
                    vps[:],
                )

            def make_proj_parts(j):
                xtile = xp.tile([P, CE // P, 512], BF16, tag="xtile")
                state = {}

                def a():
                    state["qkps"] = proj_A(j, xtile)

                def b():
                    state["vps"] = proj_B(j, xtile)

                def c():
                    proj_C(j, state["qkps"], state["vps"])

                return [a, b, c]

            def emit_proj(j):
                for fn in make_proj_parts(j):
                    fn()

            # ---- scores for pair j: blocks 2j (even chain, PE rows 0-63)
            # and 2j+1 (odd chain, rows 64-127), interleaved by group ----
            def emit_scores_pair(j, parts=(), fillers=False):
                parts = list(parts)
                i0, i1 = 2 * j, 2 * j + 1
                info = []
                for chain, i in ((0, i0), (1, i1)):
                    qlo = 256 * i
                    L = T - qlo
                    info.append((chain, i, qlo, L, ceil(L / ETILE)))
                ngmax = max(x[4] for x in info)
                for g in range(ngmax):
                    for chain, i, qlo, L, ng in info:
                        if g >= ng:
                            continue
                        gw = min(ETILE, L - ETILE * g)
                        pool = spe if chain == 0 else spo
                        sc = pool.tile([P, ETILE], F32, tag="sce" if chain == 0 else "sco")
                        rows = slice(0, CH) if chain == 0 else slice(CH, P)
                        lhs = kT2[rows, P * j : P * (j + 1)]
                        nsub = ceil(gw / 512)
                        for u in range(nsub):
                            wu = min(512, gw - 512 * u)
                            qs = qlo + ETILE * g + 512 * u
                            has_diag = (g == 0 and u == 0)
                            has_tail = (g == ng - 1 and u == nsub - 1)
                            nc.tensor.matmul(
                                sc[:, 512 * u : 512 * u + wu],
                                lhs,
                                qT2[rows, qs : qs + wu],
                                start=True,
                                stop=not (has_diag or has_tail),
                                skip_group_check=True,
                            )
                            if has_diag:
                                nc.tensor.matmul(
                                    sc[:, 0:P],
                                    atri[:],
                                    bneg[:],
                                    start=False,
                                    stop=not has_tail,
                                    skip_group_check=True,
                                )
                            if has_tail:
                                nc.tensor.matmul(
                                    sc[:, gw - P : gw],
                                    ones[:],
                                    tmask[:],
                                    start=False,
                                    stop=True,
                                    skip_group_check=True,
                                )
                        woff = 256 * (i % 2)
                        if j == 0:
                            nc.scalar.activation(
                                w2[i][:, woff + ETILE * g : woff + ETILE * g + gw],
                                sc[:, :gw],
                                mybir.ActivationFunctionType.Exp,
                                scale=SCALE,
                                accum_out=stats[:, i, g : g + 1],
                            )
                        else:
                            nc.scalar.activation(
                                w2[i][:, woff + ETILE * g : woff + ETILE * g + gw],
                                sc[:, :gw],
                                mybir.ActivationFunctionType.Exp,
                                scale=SCALE,
                            )
                    if parts:
                        parts.pop(0)()
                    if fillers:
                        fps = ppv.tile([P, P], F32, tag="vps", name=f"fill{j}_{g}")
                        nc.tensor.matmul(
                            fps[:], ones[:], ones[:], start=True, stop=True,
                            skip_group_check=True,
                        )
                for fn in parts:
                    fn()
                # per-block normalizers + scaled v
                for chain, i, qlo, L, ng in info:
                    if j == 0:
                        nc.vector.reduce_sum(
                            ssum[:, i : i + 1],
                            stats[:, i, 0:ng],
                            axis=mybir.AxisListType.X,
                        )
                    else:
                        nc.vector.reduce_sum(
                            ssum[:, i : i + 1],
                            w2[i][:],
                            axis=mybir.AxisListType.X,
                        )
                    nc.vector.reciprocal(rr[:, i : i + 1], ssum[:, i : i + 1])
                    nc.vector.tensor_scalar_mul(
                        vsc[:, i, :], vnat[:, i, :], rr[:, i : i + 1]
                    )

            # ---- output batch k = pairs {2k+1, 2k}, regions 2k..7 ----
            def emit_output_batch(k):
                hi, lo = 2 * k + 1, 2 * k
                final = k == 0
                for t in range(2 * k, NP):
                    o = 512 * t
                    po = op.tile([P, 512], F32, tag="po", name=f"b{k}t{t}")
                    plist = [p for p in (hi, lo) if p <= t]
                    for n, p in enumerate(plist):
                        off = 512 * (t - p)
                        nc.tensor.matmul(
                            po[0:CH, :],
                            vsc[:, 2 * p, :],
                            w2[2 * p][:, off : off + 512],
                            start=(n == 0),
                            stop=(n == len(plist) - 1),
                            skip_group_check=True,
                        )
                        nc.tensor.matmul(
                            po[CH:P, :],
                            vsc[:, 2 * p + 1, :],
                            w2[2 * p + 1][:, off : off + 512],
                            start=(n == 0),
                            stop=(n == len(plist) - 1),
                            skip_group_check=True,
                        )
                    if final:
                        ot = osb.tile([P, 512], BF16, tag="ot")
                        if t >= 2:
                            nc.vector.scalar_tensor_tensor(
                                ot[:],
                                po[:],
                                1.0,
                                outacc[:, o : o + 512],
                                mybir.AluOpType.bypass,
                                mybir.AluOpType.add,
                            )
                        else:
                            nc.vector.tensor_copy(ot[:], po[:])
                        dma_eng = nc.sync if t % 2 == 0 else nc.gpsimd
                        dma_eng.dma_start(outT[:, o : o + 512], ot[:])
                    elif k == t // 2:
                        nc.vector.tensor_copy(outacc[:, o : o + 512], po[:])
                    else:
                        nc.vector.scalar_tensor_tensor(
                            outacc[:, o : o + 512],
                            po[:],
                            1.0,
                            outacc[:, o : o + 512],
                            mybir.AluOpType.bypass,
                            mybir.AluOpType.add,
                        )

            # ======== pipeline: pairs descending ========
            emit_proj(NP - 1)
            for i in range(1, NB, 2):
                nc.gpsimd.memset(w2[i][:, 0:256], 0.0)
            emit_proj(NP - 2)
            for j in reversed(range(NP)):
                if j == NP - 1:
                    emit_scores_pair(j)
                    emit_proj(j - 2)
                elif j in (5, 3):
                    emit_scores_pair(j, parts=make_proj_parts(j - 2))
                elif j in (6, 4, 2):
                    emit_scores_pair(j)
                    if j >= 2:
                        emit_proj(j - 2)
                else:  # j in (1, 0): no proj left; keep HAM warm
                    emit_scores_pair(j, fillers=True)
                # output batches, lagged one pair past their last dependency
                if j == 5:
                    emit_output_batch(3)
                elif j == 3:
                    emit_output_batch(2)
                elif j == 1:
                    emit_output_batch(1)
            emit_output_batch(0)

    return nc


_PROGRAM = None


def _get_program():
    global _PROGRAM
    if _PROGRAM is None:
        nc = _build_program()
        nc.finalize()
        _PROGRAM = nc
    return _PROGRAM


def kernel(x, Wk, Wq, Wv, trace=False, trace_cores=None):
    global LAST_RESULTS
    x = np.asarray(x)
    Wk = np.asarray(Wk)
    Wq = np.asarray(Wq)
    Wv = np.asarray(Wv)

    import ml_dtypes

    bf = ml_dtypes.bfloat16
    wqk_b = np.concatenate([Wq, Wk], axis=1).astype(bf)
    wv_b = Wv.astype(bf)

    zeros_mask = np.zeros((P, P), bf)
    neg_mask = np.full((P, P), NEG / P, bf)

    in_maps = []
    for c in range(N_CORES):
        b, parity = c // 2, c % 2
        xTb = np.ascontiguousarray(x[b].T).astype(bf)  # [CE, T]
        if parity:
            xTb = np.concatenate([xTb[:, P:], np.zeros((CE, P), bf)], axis=1)
        xpre = (
            xTb.reshape(CE // P, P, NP, 512)
            .transpose(2, 1, 0, 3)
            .reshape(NP, P, CE // P * 512)
        )
        in_maps.append(
            {
                "xpre": np.ascontiguousarray(xpre),
                "wqk": wqk_b,
                "wv": wv_b,
                "tailmask": neg_mask if parity else zeros_mask,
            }
        )

    nc = _get_program()
    res = run_bass_kernel_spmd(
        nc,
        in_maps,
        list(range(N_CORES)),
        trace=trace,
        **({"trace_cores": trace_cores} if trace_cores is not None else {}),
    )
    LAST_RESULTS = res

    out = np.zeros((B, T, CH), np.float32)
    for c in range(N_CORES):
        b, parity = c // 2, c % 2
        oTf = np.asarray(res.results[c]["outT"]).astype(np.float32)  # [128, T]
        oT = oTf[0:CH] + oTf[CH:P]  # fold even/odd block halves
        if parity:
            out[b, P:, :] += oT[:, : T - P].T
        else:
            out[b] += oT.T
    return out


# revision 3
# speedup vs baseline: 1.0394x; 1.0394x over previous
"""Causal self-attention head (softmax over the QUERY axis) on 8 trn2 cores.

Reference math (softmax axis=-2, i.e. per key-column):
    q = x @ Wq; k = x @ Wk; v = x @ Wv            # [B,T,64]
    s[b,q,k] = (q . k) * 64**-0.5, masked to q >= k
    w[:, k]  = softmax over q of s[:, k]           # column softmax
    out[b,q,:] = sum_k w[q,k] v[k,:]

The softmax normalizes over q, so the normalizer folds into per-key scaling:
    out[q] = sum_{k<=q} exp(s[q,k]) * (r[k] * v[k]),  r[k] = 1/sum_{q>=k} exp(s[q,k])

Sharding: 8 cores = 4 batches x 2 "parities". Core (b, p) owns key blocks
2i+p (128 keys each); parity-1 cores get x^T pre-shifted by 128 cols
(zero-pad tail killed by a tailmask matmul); host folds + shifts output back.

v4 kernel structure (per core, pairs j = 7..0, pair = key blocks 2j/2j+1):
- proj: ONE fused [Wk||Wq] matmul per contraction subtile: psum rows 0-63 =
  k, rows 64-127 = q, for all 512 chunk cols. k's own 2x128 key cols go to
  kTl (parts 0-63) with one strided DVE copy; q is staged at parts 64-127
  and moved down to qT (parts 0-63) by a small SBUF->SBUF DMA on the gpsimd
  queue (so the sync queue only carries the big x chunk streams).
- v projected directly into natural [key, ch] layout (lhsT = x key cols).
- scores: K=64 M=128 matmuls into [128,1024] double-buffered psum groups;
  causal diag via a triangular-count matmul; exp on ACT.
- colsums: ACT accum_out for the late (big) blocks 0-7 so the tail never
  waits on a long DVE reduce; one DVE reduce over w2 for early blocks 8-15.
- output: regions accumulate TWO pairs per psum tile (batch k = pairs
  {2k+1,2k}, col-tiled even/odd halves at PE tiles (0,0)/(0,64)), then one
  DVE op into the fp32 outacc; the final batch emits bf16 and streams the
  DMA out. Dummy matmuls during pairs 1/0 keep the HAM clock-gate warm.
"""

import os
import sys
from math import ceil

import numpy as np

for _p in ("/opt/trn_rl_repo",):
    if _p not in sys.path:
        sys.path.insert(0, _p)

import concourse.bass as bass
import concourse.mybir as mybir
from concourse import bacc
from concourse.bass_utils import run_bass_kernel_spmd
from concourse.tile import TileContext

B, T, CE, CH = 4, 4096, 1024, 64
P = 128
NB = 16          # key blocks per core (128 keys each)
NP = 8           # pairs (512-col chunks)
SCALE = CH ** -0.5
NEG = -1e30
M0 = NEG / P     # per-unit magnitude for the triangular-count mask
ETILE = 1024     # scores psum group width (2 banks)

F32 = mybir.dt.float32
BF16 = mybir.dt.bfloat16

N_CORES = 8

LAST_RESULTS = None


def _build_program():
    nc = bacc.Bacc("TRN2", target_bir_lowering=False, debug=False)

    # x pre-permuted on the host: xpre[j, p, o*512+f] = x^T[o*128+p, 512j+f]
    xpre = nc.declare_dram_parameter("xpre", [NP, P, CE // P * 512], BF16, isOutput=False)
    wkq = nc.declare_dram_parameter("wkq", [CE, P], BF16, isOutput=False)
    wv = nc.declare_dram_parameter("wv", [CE, CH], BF16, isOutput=False)
    tailmask = nc.declare_dram_parameter("tailmask", [P, P], BF16, isOutput=False)
    outT = nc.declare_dram_parameter("outT", [P, T], BF16, isOutput=True)

    with TileContext(nc) as tc:
        with (
            tc.tile_pool(name="consts", bufs=1) as consts,
            tc.tile_pool(name="qkv", bufs=1) as qkv,
            tc.tile_pool(name="w2p", bufs=1) as w2p,
            tc.tile_pool(name="xp", bufs=3) as xp,
            tc.tile_pool(name="qsp", bufs=2) as qsp,
            tc.tile_pool(name="osb", bufs=2) as osb,
            tc.tile_pool(name="ppqk", bufs=1, space="PSUM") as ppqk,
            tc.tile_pool(name="ppv", bufs=1, space="PSUM") as ppv,
            tc.tile_pool(name="sp", bufs=2, space="PSUM") as sp,
            tc.tile_pool(name="op", bufs=2, space="PSUM") as op,
        ):
            # ---- gpsimd-built mask constants (ones FIRST: warmup dep) ----
            ones = consts.tile([P, P], BF16, tag="ones")
            nc.gpsimd.memset(ones[:], 1.0)
            atri = consts.tile([P, P], BF16, tag="atri")
            nc.gpsimd.memset(atri[:], 1.0)
            nc.gpsimd.affine_select(
                out=atri[:],
                in_=atri[:],
                compare_op=mybir.AluOpType.is_ge,
                fill=0.0,
                base=-1,
                pattern=[[1, P]],
                channel_multiplier=-1,
            )
            bneg = consts.tile([P, P], BF16, tag="bneg")
            nc.gpsimd.memset(bneg[:], M0)
            nc.gpsimd.affine_select(
                out=bneg[:],
                in_=bneg[:],
                compare_op=mybir.AluOpType.is_ge,
                fill=0.0,
                base=0,
                pattern=[[-1, P]],
                channel_multiplier=1,
            )

            # ---- DMA'd constants ----
            wkq_sb = consts.tile([P, CE // P, P], BF16, tag="wkq")
            wv_sb = consts.tile([P, CE // P, CH], BF16, tag="wv")
            nc.sync.dma_start(wkq_sb[:], wkq.rearrange("(o p) f -> p o f", p=P))
            nc.sync.dma_start(wv_sb[:], wv.rearrange("(o p) f -> p o f", p=P))
            tmask = consts.tile([P, P], BF16, tag="tmask")
            nc.sync.dma_start(tmask[:], tailmask[:])

            # ---- persistent activations ----
            qT = qkv.tile([CH, T], BF16, tag="qT")         # q at parts 0-63
            kTl = qkv.tile([CH, NB * P], BF16, tag="kTl")  # k blocks at parts 0-63
            vnat = qkv.tile([P, NB, CH], F32, tag="vnat")
            vsc = qkv.tile([P, NB, CH], BF16, tag="vsc")
            stats = qkv.tile([P, 8, 4], F32, tag="stats")  # accum_out, blocks 0-7
            ssum = qkv.tile([P, NB], F32, tag="ssum")
            rr = qkv.tile([P, NB], F32, tag="rr")
            outacc = qkv.tile([P, T], F32, tag="outacc")

            w2 = [
                w2p.tile([P, T - 512 * (i // 2)], BF16, tag=f"w2_{i}", name=f"w2_{i}")
                for i in range(NB)
            ]

            # PE warm-up while the first DMAs land (ones is the first gpsimd
            # memset, so this starts as soon as the preamble ends).
            for t in range(28):
                dscr = op.tile([P, 512], F32, tag="po", name=f"warm{t}")
                nc.tensor.matmul(
                    dscr[:, 0:P], ones[:, 0:P], ones[:, 0:P],
                    start=True, stop=True,
                )
            dscr = op.tile([P, 512], F32, tag="po", name="abs_tm")
            nc.tensor.matmul(
                dscr[0:1, 0:1], tmask[:, 0:1], tmask[:, 0:1],
                start=True, stop=True,
            )

            # ---- projection for chunk j, split into 3 emission parts ----
            def proj_A(j, xtile):
                nc.sync.dma_start(
                    xtile[:, 0:4, :],
                    xpre[j, :, 0 : 4 * 512].rearrange("p (o f) -> p o f", o=4),
                )
                nc.gpsimd.dma_start(
                    xtile[:, 4:8, :],
                    xpre[j, :, 4 * 512 : 8 * 512].rearrange("p (o f) -> p o f", o=4),
                )
                dscr = op.tile([P, 512], F32, tag="po", name=f"absx{j}")
                nc.tensor.matmul(
                    dscr[0:1, 0:1],
                    xtile[:, 0, 0:1],
                    xtile[:, 0, 0:1],
                    start=True,
                    stop=True,
                )
                qkps = ppqk.tile([P, 512], F32, tag="qkps")
                for s in range(CE // P):
                    nc.tensor.matmul(
                        qkps[:],
                        wkq_sb[:, s, :],
                        xtile[:, s, :],
                        start=(s == 0),
                        stop=(s == CE // P - 1),
                    )
                return qkps

            def proj_B(j, xtile):
                vps = ppv.tile([P, P], F32, tag="vps", name=f"vps{j}")
                for s in range(CE // P):
                    nc.tensor.matmul(
                        vps[:, 0:CH],
                        xtile[:, s, 0:P],
                        wv_sb[:, s, :],
                        start=(s == 0),
                        stop=(s == CE // P - 1),
                        skip_group_check=True,
                    )
                for s in range(CE // P):
                    nc.tensor.matmul(
                        vps[:, CH:P],
                        xtile[:, s, 256 : 256 + P],
                        wv_sb[:, s, :],
                        start=(s == 0),
                        stop=(s == CE // P - 1),
                        skip_group_check=True,
                    )
                return vps

            def proj_C(j, qkps, vps):
                # k rows 0-63: own key cols {0:128, 256:384} in one strided copy
                nc.vector.tensor_copy(
                    kTl[:, 256 * j : 256 * (j + 1)].rearrange(
                        "p (b c) -> p b c", c=P
                    ),
                    qkps[0:CH, 0:384].rearrange("p (b c) -> p b c", c=P)[:, 0::2, :],
                )
                # q rows 64-127: stage, then move down to qT on the gpsimd queue
                qstg = qsp.tile([P, 512], BF16, tag="qstg")
                nc.vector.tensor_copy(qstg[CH:P, :], qkps[CH:P, :])
                nc.gpsimd.dma_start(
                    qT[:, 512 * j : 512 * (j + 1)], qstg[CH:P, :]
                )
                # v blocks -> natural layout
                nc.vector.tensor_copy(
                    vnat[:, 2 * j : 2 * j + 2, :].rearrange("p b c -> p (b c)"),
                    vps[:],
                )

            def make_proj_parts(j):
                xtile = xp.tile([P, CE // P, 512], BF16, tag="xtile")
                state = {}

                def a():
                    state["qkps"] = proj_A(j, xtile)

                def b():
                    state["vps"] = proj_B(j, xtile)

                def c():
                    proj_C(j, state["qkps"], state["vps"])

                return [a, b, c]

            def emit_proj(j):
                for fn in make_proj_parts(j):
                    fn()

            # ---- scores for block i (q cols qlo..T into w2[i]) ----
            def emit_block(i):
                j = i // 2
                qlo = 256 * i
                L = T - qlo
                woff = 256 * (i % 2)
                lhs = kTl[:, P * i : P * (i + 1)]
                ngr = ceil(L / ETILE)
                for g in range(ngr):
                    gw = min(ETILE, L - ETILE * g)
                    sc = sp.tile([P, ETILE], F32, tag="sc")
                    nsub = ceil(gw / 512)
                    for u in range(nsub):
                        wu = min(512, gw - 512 * u)
                        qs = qlo + ETILE * g + 512 * u
                        has_diag = (g == 0 and u == 0)
                        has_tail = (g == ngr - 1 and u == nsub - 1)
                        nc.tensor.matmul(
                            sc[:, 512 * u : 512 * u + wu],
                            lhs,
                            qT[:, qs : qs + wu],
                            start=True,
                            stop=not (has_diag or has_tail),
                            skip_group_check=True,
                        )
                        if has_diag:
                            nc.tensor.matmul(
                                sc[:, 0:P],
                                atri[:],
                                bneg[:],
                                start=False,
                                stop=not has_tail,
                                skip_group_check=True,
                            )
                        if has_tail:
                            nc.tensor.matmul(
                                sc[:, gw - P : gw],
                                ones[:],
                                tmask[:],
                                start=False,
                                stop=True,
                                skip_group_check=True,
                            )
                    if i < 8:
                        nc.scalar.activation(
                            w2[i][:, woff + ETILE * g : woff + ETILE * g + gw],
                            sc[:, :gw],
                            mybir.ActivationFunctionType.Exp,
                            scale=SCALE,
                            accum_out=stats[:, i, g : g + 1],
                        )
                    else:
                        nc.scalar.activation(
                            w2[i][:, woff + ETILE * g : woff + ETILE * g + gw],
                            sc[:, :gw],
                            mybir.ActivationFunctionType.Exp,
                            scale=SCALE,
                        )
                if i < 8:
                    nc.vector.reduce_sum(
                        ssum[:, i : i + 1],
                        stats[:, i, 0:ngr],
                        axis=mybir.AxisListType.X,
                    )
                else:
                    nc.vector.reduce_sum(
                        ssum[:, i : i + 1],
                        w2[i][:],
                        axis=mybir.AxisListType.X,
                    )
                nc.vector.reciprocal(rr[:, i : i + 1], ssum[:, i : i + 1])
                nc.vector.tensor_scalar_mul(
                    vsc[:, i, :], vnat[:, i, :], rr[:, i : i + 1]
                )

            def emit_scores_pair(j, parts=(), fillers=False):
                parts = list(parts)
                for n, i in enumerate((2 * j, 2 * j + 1)):
                    emit_block(i)
                    if parts:
                        parts.pop(0)()
                    if fillers:
                        fps = ppv.tile([P, P], F32, tag="vps", name=f"fill{j}_{n}")
                        nc.tensor.matmul(
                            fps[:], ones[:], ones[:], start=True, stop=True,
                            skip_group_check=True,
                        )
                for fn in parts:
                    fn()

            # ---- output batch k = pairs {2k+1, 2k}, regions 2k..7 ----
            def emit_output_batch(k):
                hi, lo = 2 * k + 1, 2 * k
                final = k == 0
                for t in range(2 * k, NP):
                    o = 512 * t
                    po = op.tile([P, 512], F32, tag="po", name=f"b{k}t{t}")
                    plist = [p for p in (hi, lo) if p <= t]
                    for n, p in enumerate(plist):
                        off = 512 * (t - p)
                        nc.tensor.matmul(
                            po[0:CH, :],
                            vsc[:, 2 * p, :],
                            w2[2 * p][:, off : off + 512],
                            start=(n == 0),
                            stop=(n == len(plist) - 1),
                            skip_group_check=True,
                        )
                        nc.tensor.matmul(
                            po[CH:P, :],
                            vsc[:, 2 * p + 1, :],
                            w2[2 * p + 1][:, off : off + 512],
                            start=(n == 0),
                            stop=(n == len(plist) - 1),
                            skip_group_check=True,
                        )
                    if final:
                        ot = osb.tile([P, 512], BF16, tag="ot")
                        if t >= 2:
                            nc.vector.scalar_tensor_tensor(
                                ot[:],
                                po[:],
                                1.0,
                                outacc[:, o : o + 512],
                                mybir.AluOpType.bypass,
                                mybir.AluOpType.add,
                            )
                        else:
                            nc.vector.tensor_copy(ot[:], po[:])
                        dma_eng = nc.sync if t % 2 == 0 else nc.gpsimd
                        dma_eng.dma_start(outT[:, o : o + 512], ot[:])
                    elif k == t // 2:
                        nc.vector.tensor_copy(outacc[:, o : o + 512], po[:])
                    else:
                        nc.vector.scalar_tensor_tensor(
                            outacc[:, o : o + 512],
                            po[:],
                            1.0,
                            outacc[:, o : o + 512],
                            mybir.AluOpType.bypass,
                            mybir.AluOpType.add,
                        )

            # ======== pipeline: pairs descending, proj 2 chunks ahead ========
            emit_proj(NP - 1)
            for i in range(1, NB, 2):
                nc.gpsimd.memset(w2[i][:, 0:256], 0.0)
            emit_proj(NP - 2)
            for j in reversed(range(NP)):
                if j == NP - 1:
                    emit_scores_pair(j)
                    emit_proj(j - 2)
                elif j in (5, 3):
                    emit_scores_pair(j, parts=make_proj_parts(j - 2))
                elif j in (6, 4, 2):
                    emit_scores_pair(j)
                    if j >= 2:
                        emit_proj(j - 2)
                else:  # j in (1, 0): no proj left; keep HAM warm
                    emit_scores_pair(j, fillers=True)
                if j == 5:
                    emit_output_batch(3)
                elif j == 3:
                    emit_output_batch(2)
                elif j == 1:
                    emit_output_batch(1)
            emit_output_batch(0)

    return nc


_PROGRAM = None


def _get_program():
    global _PROGRAM
    if _PROGRAM is None:
        nc = _build_program()
        nc.finalize()
        _PROGRAM = nc
    return _PROGRAM


def kernel(x, Wk, Wq, Wv, trace=False, trace_cores=None):
    global LAST_RESULTS
    x = np.asarray(x)
    Wk = np.asarray(Wk)
    Wq = np.asarray(Wq)
    Wv = np.asarray(Wv)

    import ml_dtypes

    bf = ml_dtypes.bfloat16
    wkq_b = np.concatenate([Wk, Wq], axis=1).astype(bf)
    wv_b = Wv.astype(bf)

    zeros_mask = np.zeros((P, P), bf)
    neg_mask = np.full((P, P), NEG / P, bf)

    in_maps = []
    for c in range(N_CORES):
        b, parity = c // 2, c % 2
        xTb = np.ascontiguousarray(x[b].T).astype(bf)  # [CE, T]
        if parity:
            xTb = np.concatenate([xTb[:, P:], np.zeros((CE, P), bf)], axis=1)
        xpre = (
            xTb.reshape(CE // P, P, NP, 512)
            .transpose(2, 1, 0, 3)
            .reshape(NP, P, CE // P * 512)
        )
        in_maps.append(
            {
                "xpre": np.ascontiguousarray(xpre),
                "wkq": wkq_b,
                "wv": wv_b,
                "tailmask": neg_mask if parity else zeros_mask,
            }
        )

    nc = _get_program()
    res = run_bass_kernel_spmd(
        nc,
        in_maps,
        list(range(N_CORES)),
        trace=trace,
        **({"trace_cores": trace_cores} if trace_cores is not None else {}),
    )
    LAST_RESULTS = res

    out = np.zeros((B, T, CH), np.float32)
    for c in range(N_CORES):
        b, parity = c // 2, c % 2
        oTf = np.asarray(res.results[c]["outT"]).astype(np.float32)  # [128, T]
        oT = oTf[0:CH] + oTf[CH:P]  # fold even/odd block halves
        if parity:
            out[b, P:, :] += oT[:, : T - P].T
        else:
            out[b] += oT.T
    return out


# revision 5
# speedup vs baseline: 1.0779x; 1.0371x over previous
"""Causal self-attention head (softmax over the QUERY axis) on 8 trn2 cores.

Reference math (softmax axis=-2, i.e. per key-column):
    q = x @ Wq; k = x @ Wk; v = x @ Wv            # [B,T,64]
    s[b,q,k] = (q . k) * 64**-0.5, masked to q >= k
    w[:, k]  = softmax over q of s[:, k]           # column softmax
    out[b,q,:] = sum_k w[q,k] v[k,:]

The softmax normalizes over q, so the normalizer folds into per-key scaling:
    out[q] = sum_{k<=q} exp(s[q,k]) * (r[k] * v[k]),  r[k] = 1/sum_{q>=k} exp(s[q,k])

Sharding: 8 cores = 4 batches x 2 "parities". Core (b, p) owns key blocks
2i+p (128 keys each); parity-1 cores get x^T pre-shifted by 128 cols
(zero-pad tail killed by a tailmask matmul); host folds + shifts output back.

v4 kernel structure (per core, pairs j = 7..0, pair = key blocks 2j/2j+1):
- proj: ONE fused [Wk||Wq] matmul per contraction subtile: psum rows 0-63 =
  k, rows 64-127 = q, for all 512 chunk cols. k's own 2x128 key cols go to
  kTl (parts 0-63) with one strided DVE copy; q is staged at parts 64-127
  and moved down to qT (parts 0-63) by a small SBUF->SBUF DMA on the gpsimd
  queue (so the sync queue only carries the big x chunk streams).
- v projected directly into natural [key, ch] layout (lhsT = x key cols).
- scores: K=64 M=128 matmuls into [128,1024] double-buffered psum groups;
  causal diag via a triangular-count matmul; exp on ACT.
- colsums: ACT accum_out for the late (big) blocks 0-7 so the tail never
  waits on a long DVE reduce; one DVE reduce over w2 for early blocks 8-15.
- output: regions accumulate TWO pairs per psum tile (batch k = pairs
  {2k+1,2k}, col-tiled even/odd halves at PE tiles (0,0)/(0,64)), then one
  DVE op into the fp32 outacc; the final batch emits bf16 and streams the
  DMA out. Dummy matmuls during pairs 1/0 keep the HAM clock-gate warm.
"""

import os
import sys
from math import ceil

import numpy as np

for _p in ("/opt/trn_rl_repo",):
    if _p not in sys.path:
        sys.path.insert(0, _p)

import concourse.bass as bass
import concourse.mybir as mybir
from concourse import bacc
from concourse.bass_utils import run_bass_kernel_spmd
from concourse.tile import TileContext

B, T, CE, CH = 4, 4096, 1024, 64
P = 128
NB = 16          # key blocks per core (128 keys each)
NP = 8           # pairs (512-col chunks)
SCALE = CH ** -0.5
NEG = -1e30
M0 = NEG / P     # per-unit magnitude for the triangular-count mask
ETILE = 1024     # scores psum group width (2 banks)

F32 = mybir.dt.float32
BF16 = mybir.dt.bfloat16

N_CORES = 8

LAST_RESULTS = None


def _build_program():
    nc = bacc.Bacc("TRN2", target_bir_lowering=False, debug=False)

    # x pre-permuted on the host: xpre[j, p, o*512+f] = x^T[o*128+p, 512j+f]
    xpre = nc.declare_dram_parameter("xpre", [NP, P, CE // P * 512], BF16, isOutput=False)
    wkq = nc.declare_dram_parameter("wkq", [CE, P], BF16, isOutput=False)
    wv = nc.declare_dram_parameter("wv", [CE, CH], BF16, isOutput=False)
    tailmask = nc.declare_dram_parameter("tailmask", [P, P], BF16, isOutput=False)
    outT = nc.declare_dram_parameter("outT", [P, T], BF16, isOutput=True)

    with TileContext(nc) as tc:
        with (
            tc.tile_pool(name="consts", bufs=1) as consts,
            tc.tile_pool(name="qkv", bufs=1) as qkv,
            tc.tile_pool(name="w2p", bufs=1) as w2p,
            tc.tile_pool(name="xp", bufs=3) as xp,
            tc.tile_pool(name="qsp", bufs=2) as qsp,
            tc.tile_pool(name="osb", bufs=2) as osb,
            tc.tile_pool(name="ppqk", bufs=1, space="PSUM") as ppqk,
            tc.tile_pool(name="ppv", bufs=1, space="PSUM") as ppv,
            tc.tile_pool(name="sp", bufs=2, space="PSUM") as sp,
            tc.tile_pool(name="op", bufs=2, space="PSUM") as op,
        ):
            # ---- gpsimd-built mask constants (ones FIRST: warmup dep) ----
            ones = consts.tile([P, P], BF16, tag="ones")
            nc.gpsimd.memset(ones[:], 1.0)
            atri = consts.tile([P, P], BF16, tag="atri")
            nc.gpsimd.memset(atri[:], 1.0)
            nc.gpsimd.affine_select(
                out=atri[:],
                in_=atri[:],
                compare_op=mybir.AluOpType.is_ge,
                fill=0.0,
                base=-1,
                pattern=[[1, P]],
                channel_multiplier=-1,
            )
            bneg = consts.tile([P, P], BF16, tag="bneg")
            nc.gpsimd.memset(bneg[:], M0)
            nc.gpsimd.affine_select(
                out=bneg[:],
                in_=bneg[:],
                compare_op=mybir.AluOpType.is_ge,
                fill=0.0,
                base=0,
                pattern=[[-1, P]],
                channel_multiplier=1,
            )

            # ---- chunk 7 input DMA first (critical path), then consts ----
            xtile7 = xp.tile([P, CE // P, 512], BF16, tag="xtile")
            nc.sync.dma_start(
                xtile7[:, 0:4, :],
                xpre[NP - 1, :, 0 : 4 * 512].rearrange("p (o f) -> p o f", o=4),
            )
            nc.gpsimd.dma_start(
                xtile7[:, 4:8, :],
                xpre[NP - 1, :, 4 * 512 : 8 * 512].rearrange("p (o f) -> p o f", o=4),
            )
            wkq_sb = consts.tile([P, CE // P, P], BF16, tag="wkq")
            wv_sb = consts.tile([P, CE // P, CH], BF16, tag="wv")
            nc.sync.dma_start(wkq_sb[:], wkq.rearrange("(o p) f -> p o f", p=P))
            nc.sync.dma_start(wv_sb[:], wv.rearrange("(o p) f -> p o f", p=P))
            tmask = consts.tile([P, P], BF16, tag="tmask")
            nc.sync.dma_start(tmask[:], tailmask[:])

            # ---- persistent activations ----
            qT = qkv.tile([CH, T], BF16, tag="qT")         # q at parts 0-63
            kTl = qkv.tile([CH, NB * P], BF16, tag="kTl")  # k blocks at parts 0-63
            vnat = qkv.tile([P, NB, CH], F32, tag="vnat")
            vsc = qkv.tile([P, NB, CH], BF16, tag="vsc")
            stats = qkv.tile([P, 8, 4], F32, tag="stats")  # accum_out, blocks 0-7
            ssum = qkv.tile([P, NB], F32, tag="ssum")
            rr = qkv.tile([P, NB], F32, tag="rr")
            outacc = qkv.tile([P, T], F32, tag="outacc")

            w2 = [
                w2p.tile([P, T - 512 * (i // 2)], BF16, tag=f"w2_{i}", name=f"w2_{i}")
                for i in range(NB)
            ]

            # PE warm-up while the first DMAs land (ones is the first gpsimd
            # memset, so this starts as soon as the preamble ends).
            for t in range(12):
                dscr = op.tile([P, 512], F32, tag="po", name=f"warm{t}")
                nc.tensor.matmul(
                    dscr[:, 0:P], ones[:, 0:P], ones[:, 0:P],
                    start=True, stop=True,
                )
            dscr = op.tile([P, 512], F32, tag="po", name="abs_tm")
            nc.tensor.matmul(
                dscr[0:1, 0:1], tmask[:, 0:1], tmask[:, 0:1],
                start=True, stop=True,
            )

            # ---- projection for chunk j, split into 3 emission parts ----
            def proj_dma(j, xtile):
                nc.sync.dma_start(
                    xtile[:, 0:4, :],
                    xpre[j, :, 0 : 4 * 512].rearrange("p (o f) -> p o f", o=4),
                )
                nc.gpsimd.dma_start(
                    xtile[:, 4:8, :],
                    xpre[j, :, 4 * 512 : 8 * 512].rearrange("p (o f) -> p o f", o=4),
                )

            def proj_A(j, xtile):
                dscr = op.tile([P, 512], F32, tag="po", name=f"absx{j}")
                nc.tensor.matmul(
                    dscr[0:1, 0:1],
                    xtile[:, 0, 0:1],
                    xtile[:, 0, 0:1],
                    start=True,
                    stop=True,
                )
                qkps = ppqk.tile([P, 512], F32, tag="qkps")
                for s in range(CE // P):
                    nc.tensor.matmul(
                        qkps[:],
                        wkq_sb[:, s, :],
                        xtile[:, s, :],
                        start=(s == 0),
                        stop=(s == CE // P - 1),
                    )
                return qkps

            def proj_B(j, xtile):
                vps = ppv.tile([P, P], F32, tag="vps", name=f"vps{j}")
                for s in range(CE // P):
                    nc.tensor.matmul(
                        vps[:, 0:CH],
                        xtile[:, s, 0:P],
                        wv_sb[:, s, :],
                        start=(s == 0),
                        stop=(s == CE // P - 1),
                        skip_group_check=True,
                    )
                for s in range(CE // P):
                    nc.tensor.matmul(
                        vps[:, CH:P],
                        xtile[:, s, 256 : 256 + P],
                        wv_sb[:, s, :],
                        start=(s == 0),
                        stop=(s == CE // P - 1),
                        skip_group_check=True,
                    )
                return vps

            def proj_C(j, qkps, vps):
                # k rows 0-63: own key cols {0:128, 256:384} in one strided copy
                nc.vector.tensor_copy(
                    kTl[:, 256 * j : 256 * (j + 1)].rearrange(
                        "p (b c) -> p b c", c=P
                    ),
                    qkps[0:CH, 0:384].rearrange("p (b c) -> p b c", c=P)[:, 0::2, :],
                )
                # q rows 64-127: stage, then move down to qT on the gpsimd queue
                qstg = qsp.tile([P, 512], BF16, tag="qstg")
                nc.vector.tensor_copy(qstg[CH:P, :], qkps[CH:P, :])
                nc.scalar.dma_start(
                    qT[:, 512 * j : 512 * (j + 1)], qstg[CH:P, :]
                )
                # v blocks -> natural layout
                nc.vector.tensor_copy(
                    vnat[:, 2 * j : 2 * j + 2, :].rearrange("p b c -> p (b c)"),
                    vps[:],
                )

            def make_proj_parts(j, xtile=None):
                if xtile is None:
                    xtile = xp.tile([P, CE // P, 512], BF16, tag="xtile")
                    proj_dma(j, xtile)
                state = {}

                def a():
                    state["qkps"] = proj_A(j, xtile)

                def b():
                    state["vps"] = proj_B(j, xtile)

                def c():
                    proj_C(j, state["qkps"], state["vps"])

                return [a, b, c]

            def emit_proj(j):
                for fn in make_proj_parts(j):
                    fn()

            # ---- scores for block i (q cols qlo..T into w2[i]) ----
            def emit_block(i):
                j = i // 2
                qlo = 256 * i
                L = T - qlo
                woff = 256 * (i % 2)
                lhs = kTl[:, P * i : P * (i + 1)]
                ngr = ceil(L / ETILE)
                for g in range(ngr):
                    gw = min(ETILE, L - ETILE * g)
                    sc = sp.tile([P, ETILE], F32, tag="sc")
                    nsub = ceil(gw / 512)
                    for u in range(nsub):
                        wu = min(512, gw - 512 * u)
                        qs = qlo + ETILE * g + 512 * u
                        has_diag = (g == 0 and u == 0)
                        has_tail = (g == ngr - 1 and u == nsub - 1)
                        nc.tensor.matmul(
                            sc[:, 512 * u : 512 * u + wu],
                            lhs,
                            qT[:, qs : qs + wu],
                            start=True,
                            stop=not (has_diag or has_tail),
                            skip_group_check=True,
                        )
                        if has_diag:
                            nc.tensor.matmul(
                                sc[:, 0:P],
                                atri[:],
                                bneg[:],
                                start=False,
                                stop=not has_tail,
                                skip_group_check=True,
                            )
                        if has_tail:
                            nc.tensor.matmul(
                                sc[:, gw - P : gw],
                                ones[:],
                                tmask[:],
                                start=False,
                                stop=True,
                                skip_group_check=True,
                            )
                    if i < 8:
                        nc.scalar.activation(
                            w2[i][:, woff + ETILE * g : woff + ETILE * g + gw],
                            sc[:, :gw],
                            mybir.ActivationFunctionType.Exp,
                            scale=SCALE,
                            accum_out=stats[:, i, g : g + 1],
                        )
                    else:
                        nc.scalar.activation(
                            w2[i][:, woff + ETILE * g : woff + ETILE * g + gw],
                            sc[:, :gw],
                            mybir.ActivationFunctionType.Exp,
                            scale=SCALE,
                        )
                if i < 8:
                    nc.vector.reduce_sum(
                        ssum[:, i : i + 1],
                        stats[:, i, 0:ngr],
                        axis=mybir.AxisListType.X,
                    )
                else:
                    nc.vector.reduce_sum(
                        ssum[:, i : i + 1],
                        w2[i][:],
                        axis=mybir.AxisListType.X,
                    )
                nc.vector.reciprocal(rr[:, i : i + 1], ssum[:, i : i + 1])
                nc.vector.tensor_scalar_mul(
                    vsc[:, i, :], vnat[:, i, :], rr[:, i : i + 1]
                )

            def emit_scores_pair(j, parts=(), fillers=False):
                parts = list(parts)
                for n, i in enumerate((2 * j, 2 * j + 1)):
                    emit_block(i)
                    if parts:
                        parts.pop(0)()
                    if fillers:
                        fps = ppv.tile([P, P], F32, tag="vps", name=f"fill{j}_{n}")
                        nc.tensor.matmul(
                            fps[:], ones[:], ones[:], start=True, stop=True,
                            skip_group_check=True,
                        )
                for fn in parts:
                    fn()

            # ---- output batch k = pairs {2k+1, 2k}, regions 2k..7 ----
            def emit_output_batch(k):
                hi, lo = 2 * k + 1, 2 * k
                final = k == 0
                for t in range(2 * k, NP):
                    o = 512 * t
                    po = op.tile([P, 512], F32, tag="po", name=f"b{k}t{t}")
                    plist = [p for p in (hi, lo) if p <= t]
                    for n, p in enumerate(plist):
                        off = 512 * (t - p)
                        nc.tensor.matmul(
                            po[0:CH, :],
                            vsc[:, 2 * p, :],
                            w2[2 * p][:, off : off + 512],
                            start=(n == 0),
                            stop=(n == len(plist) - 1),
                            skip_group_check=True,
                        )
                        nc.tensor.matmul(
                            po[CH:P, :],
                            vsc[:, 2 * p + 1, :],
                            w2[2 * p + 1][:, off : off + 512],
                            start=(n == 0),
                            stop=(n == len(plist) - 1),
                            skip_group_check=True,
                        )
                    if final:
                        ot = osb.tile([P, 512], BF16, tag="ot")
                        if t >= 2:
                            nc.vector.scalar_tensor_tensor(
                                ot[:],
                                po[:],
                                1.0,
                                outacc[:, o : o + 512],
                                mybir.AluOpType.bypass,
                                mybir.AluOpType.add,
                            )
                        else:
                            nc.vector.tensor_copy(ot[:], po[:])
                        dma_eng = nc.sync if t % 2 == 0 else nc.gpsimd
                        dma_eng.dma_start(outT[:, o : o + 512], ot[:])
                    elif k == t // 2:
                        nc.vector.tensor_copy(outacc[:, o : o + 512], po[:])
                    else:
                        nc.vector.scalar_tensor_tensor(
                            outacc[:, o : o + 512],
                            po[:],
                            1.0,
                            outacc[:, o : o + 512],
                            mybir.AluOpType.bypass,
                            mybir.AluOpType.add,
                        )

            # ======== pipeline: pairs descending, proj 2 chunks ahead ========
            # Only proj(7) precedes scores(7) in the PE FIFO; its chunk DMA
            # was already issued above, ahead of the consts DMAs.
            for fn in make_proj_parts(NP - 1, xtile=xtile7):
                fn()
            for i in range(1, NB, 2):
                nc.gpsimd.memset(w2[i][:, 0:256], 0.0)
            for j in reversed(range(NP)):
                emit_scores_pair(j, fillers=(j < 2))
                if j == NP - 1:
                    emit_proj(j - 1)
                if j >= 2:
                    emit_proj(j - 2)
                if j == 5:
                    emit_output_batch(3)
                elif j == 3:
                    emit_output_batch(2)
                elif j == 1:
                    emit_output_batch(1)
            emit_output_batch(0)

    return nc


_PROGRAM = None


def _get_program():
    global _PROGRAM
    if _PROGRAM is None:
        nc = _build_program()
        nc.finalize()
        _PROGRAM = nc
    return _PROGRAM


def kernel(x, Wk, Wq, Wv, trace=False, trace_cores=None):
    global LAST_RESULTS
    x = np.asarray(x)
    Wk = np.asarray(Wk)
    Wq = np.asarray(Wq)
    Wv = np.asarray(Wv)

    import ml_dtypes

    bf = ml_dtypes.bfloat16
    wkq_b = np.concatenate([Wk, Wq], axis=1).astype(bf)
    wv_b = Wv.astype(bf)

    zeros_mask = np.zeros((P, P), bf)
    neg_mask = np.full((P, P), NEG / P, bf)

    in_maps = []
    for c in range(N_CORES):
        b, parity = c // 2, c % 2
        xTb = np.ascontiguousarray(x[b].T).astype(bf)  # [CE, T]
        if parity:
            xTb = np.concatenate([xTb[:, P:], np.zeros((CE, P), bf)], axis=1)
        xpre = (
            xTb.reshape(CE // P, P, NP, 512)
            .transpose(2, 1, 0, 3)
            .reshape(NP, P, CE // P * 512)
        )
        in_maps.append(
            {
                "xpre": np.ascontiguousarray(xpre),
                "wkq": wkq_b,
                "wv": wv_b,
                "tailmask": neg_mask if parity else zeros_mask,
            }
        )

    nc = _get_program()
    res = run_bass_kernel_spmd(
        nc,
        in_maps,
        list(range(N_CORES)),
        trace=trace,
        **({"trace_cores": trace_cores} if trace_cores is not None else {}),
    )
    LAST_RESULTS = res

    out = np.zeros((B, T, CH), np.float32)
    for c in range(N_CORES):
        b, parity = c // 2, c % 2
        oTf = np.asarray(res.results[c]["outT"]).astype(np.float32)  # [128, T]
        oT = oTf[0:CH] + oTf[CH:P]  # fold even/odd block halves
        if parity:
            out[b, P:, :] += oT[:, : T - P].T
        else:
            out[b] += oT.T
    return out


# revision 6
# speedup vs baseline: 1.1646x; 1.0804x over previous
"""Causal self-attention head (softmax over the QUERY axis) on 8 trn2 cores.

Reference math (softmax axis=-2, i.e. per key-column):
    q = x @ Wq; k = x @ Wk; v = x @ Wv            # [B,T,64]
    s[b,q,k] = (q . k) * 64**-0.5, masked to q >= k
    w[:, k]  = softmax over q of s[:, k]           # column softmax
    out[b,q,:] = sum_k w[q,k] v[k,:]

The softmax normalizes over q, so the normalizer folds into per-key scaling:
    out[q] = sum_{k<=q} exp(s[q,k]) * (r[k] * v[k]),  r[k] = 1/sum_{q>=k} exp(s[q,k])

Sharding: 8 cores = 4 batches x 2 "parities". Core (b, p) owns key blocks
2i+p (128 keys each); parity-1 cores get x^T pre-shifted by 128 cols
(zero-pad tail killed by a tailmask matmul); host shifts output back.

v2 kernel structure (per core, pairs j = 7..0, pair = key blocks 2j/2j+1):
- proj: ONE [Wq||Wk] matmul per contraction subtile (M=128): psum rows 0-63
  = q, rows 64-127 = k, for all 512 chunk cols. One DVE cast evacuates both;
  the core's own 2x128 key cols of k are relocated to partitions 0-63 by a
  tiny SBUF->SBUF DMA (scores need lhs/rhs on the same partitions).
- v is projected directly into natural [key, ch] layout via lhsT=x-chunk,
  rhs=Wv (N=64 matmuls) -- no DMA transposes at all.
- scores: K=64 M=128 matmuls into [128,1024] psum groups; causal diag via a
  triangular-count matmul; exp on ACT with accum_out colsums (fp32).
- output: streamed per pair with COL-TILED matmul pairs (tile_position
  (0,0)/(0,64)): even-block partial in psum rows 0-63, odd in 64-127,
  concurrently (2x PE throughput). DVE accumulates into an SBUF [128,T]
  accumulator; the even/odd halves are summed on the HOST (outT is [128,T]).
- odd blocks skip their 256 dead columns (w2 zero prefix via gpsimd memset
  instead of exp of -inf).
"""

import os
import sys
from math import ceil

import numpy as np

for _p in ("/opt/trn_rl_repo",):
    if _p not in sys.path:
        sys.path.insert(0, _p)

import concourse.bass as bass
import concourse.mybir as mybir
from concourse import bacc
from concourse.bass_utils import run_bass_kernel_spmd
from concourse.tile import TileContext

B, T, CE, CH = 4, 4096, 1024, 64
P = 128
NB = 16          # key blocks per core (128 keys each)
NP = 8           # pairs (512-col chunks)
SCALE = CH ** -0.5
NEG = -1e30
M0 = NEG / P     # per-unit magnitude for the triangular-count mask
ETILE = 1024     # scores psum group width (2 banks)

F32 = mybir.dt.float32
BF16 = mybir.dt.bfloat16

N_CORES = 8

LAST_RESULTS = None


def _build_program():
    nc = bacc.Bacc("TRN2", target_bir_lowering=False, debug=False)

    # x pre-permuted on the host: xpre[j, p, o*512+f] = x^T[o*128+p, 512j+f]
    # so each 512-col chunk is a single contiguous 1 MB HBM burst.
    xpre = nc.declare_dram_parameter("xpre", [NP, P, CE // P * 512], BF16, isOutput=False)
    wq = nc.declare_dram_parameter("wq", [CE, CH], BF16, isOutput=False)
    wk = nc.declare_dram_parameter("wk", [CE, CH], BF16, isOutput=False)
    wv = nc.declare_dram_parameter("wv", [CE, CH], BF16, isOutput=False)
    tailmask = nc.declare_dram_parameter("tailmask", [P, P], BF16, isOutput=False)
    outT = nc.declare_dram_parameter("outT", [P, T], F32, isOutput=True)

    with TileContext(nc) as tc:
        with (
            tc.tile_pool(name="consts", bufs=1) as consts,
            tc.tile_pool(name="qkv", bufs=1) as qkv,
            tc.tile_pool(name="w2p", bufs=1) as w2p,
            tc.tile_pool(name="xp", bufs=3) as xp,
            tc.tile_pool(name="pp", bufs=1, space="PSUM") as pp,
            tc.tile_pool(name="sp", bufs=2, space="PSUM") as sp,
            tc.tile_pool(name="op", bufs=2, space="PSUM") as op,
        ):
            # ---- chunk 7 input DMA first (it gates the whole pipeline) ----
            xtile7 = xp.tile([P, CE // P, 512], BF16, tag="xtile")
            nc.sync.dma_start(
                xtile7[:, 0:4, :],
                xpre[NP - 1, :, 0 : 4 * 512].rearrange("p (o f) -> p o f", o=4),
            )
            nc.gpsimd.dma_start(
                xtile7[:, 4:8, :],
                xpre[NP - 1, :, 4 * 512 : 8 * 512].rearrange("p (o f) -> p o f", o=4),
            )

            # ---- DMA'd constants ----
            wq_sb = consts.tile([P, CE // P, CH], BF16, tag="wq")
            wk_sb = consts.tile([P, CE // P, CH], BF16, tag="wk")
            wv_sb = consts.tile([P, CE // P, CH], BF16, tag="wv")
            nc.sync.dma_start(wq_sb[:], wq.rearrange("(o p) f -> p o f", p=P))
            nc.sync.dma_start(wk_sb[:], wk.rearrange("(o p) f -> p o f", p=P))
            nc.sync.dma_start(wv_sb[:], wv.rearrange("(o p) f -> p o f", p=P))
            tmask = consts.tile([P, P], BF16, tag="tmask")
            nc.sync.dma_start(tmask[:], tailmask[:])

            # ---- gpsimd-built mask constants ----
            # atri[ch, p] = 1 if ch < p; bneg[ch, c] = M0 if c <= ch
            # => (atri^T @ bneg)[p, c] = M0 * max(0, p - c)
            ones = consts.tile([P, P], BF16, tag="ones")
            nc.gpsimd.memset(ones[:], 1.0)
            atri = consts.tile([P, P], BF16, tag="atri")
            nc.gpsimd.memset(atri[:], 1.0)
            nc.gpsimd.affine_select(
                out=atri[:],
                in_=atri[:],
                compare_op=mybir.AluOpType.is_ge,
                fill=0.0,
                base=-1,
                pattern=[[1, P]],
                channel_multiplier=-1,
            )
            bneg = consts.tile([P, P], BF16, tag="bneg")
            nc.gpsimd.memset(bneg[:], M0)
            nc.gpsimd.affine_select(
                out=bneg[:],
                in_=bneg[:],
                compare_op=mybir.AluOpType.is_ge,
                fill=0.0,
                base=0,
                pattern=[[-1, P]],
                channel_multiplier=1,
            )

            # ---- persistent activations ----
            qT = qkv.tile([CH, T], BF16, tag="qT")         # q at parts 0-63
            kTl = qkv.tile([CH, NB * P], BF16, tag="kTl")  # k blocks at parts 0-63
            vnat = qkv.tile([P, NB, CH], F32, tag="vnat")
            vsc = qkv.tile([P, NB, CH], BF16, tag="vsc")
            stats = qkv.tile([P, NB, 4], F32, tag="stats")
            ssum = qkv.tile([P, NB], F32, tag="ssum")
            rr = qkv.tile([P, NB], F32, tag="rr")
            outacc = qkv.tile([P, T], F32, tag="outacc")

            w2 = [
                w2p.tile([P, T - 512 * (i // 2)], BF16, tag=f"w2_{i}", name=f"w2_{i}")
                for i in range(NB)
            ]

            # A few PE warm-up matmuls while the first input DMAs land (the
            # HAM clock-gate needs sustained activity; a long spam train
            # would head-of-line-block real work, so keep it short).
            for t in range(12):
                dscr = op.tile([P, 512], F32, tag="po", name=f"warm{t}")
                nc.tensor.matmul(
                    dscr[:, 0:P], ones[:, 0:P], ones[:, 0:P],
                    start=True, stop=True,
                )
            dscr = op.tile([P, 512], F32, tag="po", name="abs_tm")
            nc.tensor.matmul(
                dscr[0:1, 0:1], tmask[:, 0:1], tmask[:, 0:1],
                start=True, stop=True,
            )

            def emit_block(i):
                j = i // 2
                odd = i % 2
                qlo = 512 * j + 256 * odd   # first live q col for this block
                L = T - qlo                  # number of exp cols
                woff = 256 * odd             # col in w2[i] where q=qlo lands
                lhs = kTl[:, P * i : P * (i + 1)]
                ngr = ceil(L / ETILE)
                for g in range(ngr):
                    gw = min(ETILE, L - ETILE * g)
                    sc = sp.tile([P, ETILE], F32, tag="sc")
                    nsub = ceil(gw / 512)
                    for u in range(nsub):
                        wu = min(512, gw - 512 * u)
                        qs = qlo + ETILE * g + 512 * u
                        # bank u gets the diag mask iff (g==0 and u==0);
                        # the tail-kill iff last group and u is last bank
                        has_diag = (g == 0 and u == 0)
                        has_tail = (g == ngr - 1 and u == nsub - 1)
                        nc.tensor.matmul(
                            sc[:, 512 * u : 512 * u + wu],
                            lhs,
                            qT[:, qs : qs + wu],
                            start=True,
                            stop=not (has_diag or has_tail),
                            skip_group_check=True,
                        )
                        if has_diag:
                            nc.tensor.matmul(
                                sc[:, 0:P],
                                atri[:],
                                bneg[:],
                                start=False,
                                stop=not has_tail,
                                skip_group_check=True,
                            )
                        if has_tail:
                            nc.tensor.matmul(
                                sc[:, gw - P : gw],
                                ones[:],
                                tmask[:],
                                start=False,
                                stop=True,
                                skip_group_check=True,
                            )
                    nc.scalar.activation(
                        w2[i][:, woff + ETILE * g : woff + ETILE * g + gw],
                        sc[:, :gw],
                        mybir.ActivationFunctionType.Exp,
                        scale=SCALE,
                        accum_out=stats[:, i, g : g + 1],
                    )
                nc.vector.reduce_sum(
                    ssum[:, i : i + 1],
                    stats[:, i, 0:ngr],
                    axis=mybir.AxisListType.X,
                )
                nc.vector.reciprocal(rr[:, i : i + 1], ssum[:, i : i + 1])
                nc.vector.tensor_scalar_mul(
                    vsc[:, i, :], vnat[:, i, :], rr[:, i : i + 1]
                )

            # ---- streamed output for pair j (col-tiled even/odd) ----
            # Emitted one iteration AFTER pair j's exp chain so the PE queue
            # (strict FIFO) never stalls on the ACT->rr->vsc dependency: by
            # the time the PE reaches these matmuls, vsc[j] is long done.
            def emit_output(j):
                for t in range(NP - j):
                    o = 512 * j + 512 * t
                    po = op.tile([P, 512], F32, tag="po", name=f"po{j}_{t}")
                    nc.tensor.matmul(
                        po[0:CH, :],
                        vsc[:, 2 * j, :],
                        w2[2 * j][:, 512 * t : 512 * t + 512],
                        start=True,
                        stop=True,
                        skip_group_check=True,
                    )
                    nc.tensor.matmul(
                        po[CH:P, :],
                        vsc[:, 2 * j + 1, :],
                        w2[2 * j + 1][:, 512 * t : 512 * t + 512],
                        start=True,
                        stop=True,
                        skip_group_check=True,
                    )
                    if t == 0:
                        nc.vector.tensor_copy(outacc[:, o : o + 512], po[:])
                    else:
                        nc.vector.scalar_tensor_tensor(
                            outacc[:, o : o + 512],
                            po[:],
                            1.0,
                            outacc[:, o : o + 512],
                            mybir.AluOpType.bypass,
                            mybir.AluOpType.add,
                        )
                    if j == 0:
                        # region o is final once pair 0 lands; stream it out
                        dma_eng = nc.sync if t % 2 == 0 else nc.gpsimd
                        dma_eng.dma_start(
                            outT[:, o : o + 512], outacc[:, o : o + 512]
                        )

            def emit_proj(j, xtile=None):
                if xtile is None:
                    xtile = xp.tile([P, CE // P, 512], BF16, tag="xtile")
                    # split the 1MB chunk across two queues for 2x transfer bw
                    nc.sync.dma_start(
                        xtile[:, 0:4, :],
                        xpre[j, :, 0 : 4 * 512].rearrange("p (o f) -> p o f", o=4),
                    )
                    nc.gpsimd.dma_start(
                        xtile[:, 4:8, :],
                        xpre[j, :, 4 * 512 : 8 * 512].rearrange("p (o f) -> p o f", o=4),
                    )
                # absorber: put this chunk's DMA wait on a throwaway MM
                dscr = op.tile([P, 512], F32, tag="po", name=f"absx{j}")
                nc.tensor.matmul(
                    dscr[0:1, 0:1],
                    xtile[:, 0, 0:1],
                    xtile[:, 0, 0:1],
                    start=True,
                    stop=True,
                )

                # q projection: [64, 512] psum at parts 0-63, straight cast
                qps = pp.tile([CH, 512], F32, tag="qps")
                for s in range(CE // P):
                    nc.tensor.matmul(
                        qps[:],
                        wq_sb[:, s, :],
                        xtile[:, s, :],
                        start=(s == 0),
                        stop=(s == CE // P - 1),
                    )
                nc.vector.tensor_copy(qT[:, 512 * j : 512 * (j + 1)], qps[:])

                # k (own 2x128 key cols) and v (natural layout) share a bank:
                # kvps[:, 0:128] = v (keys x ch for both blocks),
                # kvps[0:64, 128:384] = kT for both blocks.
                kvps = pp.tile([P, 384], F32, tag="kvps")
                for s in range(CE // P):
                    kvrhs = xtile[:, s, :].rearrange("p (b c) -> p b c", c=P)[
                        :, 0::2, :
                    ]
                    nc.tensor.matmul(
                        kvps[0:CH, 128:384].rearrange("p (b c) -> p b c", c=P),
                        wk_sb[:, s, :],
                        kvrhs,
                        start=(s == 0),
                        stop=(s == CE // P - 1),
                        skip_group_check=True,
                    )
                for s in range(CE // P):
                    nc.tensor.matmul(
                        kvps[:, 0:CH],
                        xtile[:, s, 0:P],
                        wv_sb[:, s, :],
                        start=(s == 0),
                        stop=(s == CE // P - 1),
                        skip_group_check=True,
                    )
                for s in range(CE // P):
                    nc.tensor.matmul(
                        kvps[:, CH:P],
                        xtile[:, s, 256 : 256 + P],
                        wv_sb[:, s, :],
                        start=(s == 0),
                        stop=(s == CE // P - 1),
                        skip_group_check=True,
                    )
                nc.vector.tensor_copy(
                    vnat[:, 2 * j : 2 * j + 2, :].rearrange("p b c -> p (b c)"),
                    kvps[:, 0:P],
                )
                nc.vector.tensor_copy(
                    kTl[:, 256 * j : 256 * (j + 1)], kvps[0:CH, 128:384]
                )

            # ======== pipeline: pairs descending, proj runs 2 pairs ahead ====
            # Only proj(7) precedes scores(7) in the PE FIFO (so the first
            # exp isn't blocked behind chunk 6's DMA); proj(6) and proj(5)
            # follow immediately after, restoring the 2-chunk lookahead.
            emit_proj(NP - 1, xtile=xtile7)
            # odd blocks: first 256 cols are a zero prefix (dead causal zone)
            for i in range(1, NB, 2):
                nc.gpsimd.memset(w2[i][:, 0:256], 0.0)
            for j in reversed(range(NP)):
                emit_block(2 * j)
                emit_block(2 * j + 1)
                if j == NP - 1:
                    emit_proj(j - 1)
                if j >= 2:
                    emit_proj(j - 2)
                if j < NP - 1:
                    emit_output(j + 1)
            emit_output(0)

    return nc


_PROGRAM = None


def _get_program():
    global _PROGRAM
    if _PROGRAM is None:
        nc = _build_program()
        nc.finalize()
        _PROGRAM = nc
    return _PROGRAM


def kernel(x, Wk, Wq, Wv, trace=False, trace_cores=None):
    global LAST_RESULTS
    x = np.asarray(x)
    Wk = np.asarray(Wk)
    Wq = np.asarray(Wq)
    Wv = np.asarray(Wv)

    import ml_dtypes

    bf = ml_dtypes.bfloat16
    wq_b = Wq.astype(bf)
    wk_b = Wk.astype(bf)
    wv_b = Wv.astype(bf)

    zeros_mask = np.zeros((P, P), bf)
    neg_mask = np.full((P, P), NEG / P, bf)

    in_maps = []
    for c in range(N_CORES):
        b, parity = c // 2, c % 2
        xTb = np.ascontiguousarray(x[b].T).astype(bf)  # [CE, T]
        if parity:
            xTb = np.concatenate([xTb[:, P:], np.zeros((CE, P), bf)], axis=1)
        # xpre[j, p, o*512+f] = xT[o*128+p, 512j+f]: contiguous per chunk
        xpre = (
            xTb.reshape(CE // P, P, NP, 512)
            .transpose(2, 1, 0, 3)
            .reshape(NP, P, CE // P * 512)
        )
        in_maps.append(
            {
                "xpre": np.ascontiguousarray(xpre),
                "wq": wq_b,
                "wk": wk_b,
                "wv": wv_b,
                "tailmask": neg_mask if parity else zeros_mask,
            }
        )

    nc = _get_program()
    res = run_bass_kernel_spmd(
        nc,
        in_maps,
        list(range(N_CORES)),
        trace=trace,
        **({"trace_cores": trace_cores} if trace_cores is not None else {}),
    )
    LAST_RESULTS = res

    out = np.zeros((B, T, CH), np.float32)
    for c in range(N_CORES):
        b, parity = c // 2, c % 2
        oTf = np.asarray(res.results[c]["outT"], np.float32)  # [128, T]
        oT = oTf[0:CH] + oTf[CH:P]  # fold even/odd block halves
        if parity:
            out[b, P:, :] += oT[:, : T - P].T
        else:
            out[b] += oT.T
    return out

